# revision 7
# baseline (speedup 1.0000x reference)
"""GAT attention kernel for Trainium2 (Bass/Tile), 8-core data parallel.

Problem: B=16 examples, each with 258 node embeddings of dim 64.
  e      = LayerNorm(embeddings)
  uid0   = e[:,0], iid0 = e[:,1], iatt = e[:,2:]          (N=256 items)
  ua     = uid0 * iatt                                     [B,N,D]
  scores qk_ij = LeakyReLU(s_q_i + s_k_j + s_i + a_b)  -> softmax_j -> alpha
  value_ij = LayerNorm(ua_i * ua_j)                        (never materialized!)
  att_i  = sum_j alpha_ij * value_ij
  out    = LeakyReLU(concat([uid0*iid0], att))             [B,257,D]

Key algebra: LayerNorm(x) = (x-mu)/sigma * g + b with per-pair scalars
  mu_ij  = (UA @ UA^T)_ij / D
  E2_ij  = (UA^2 @ (UA^2)^T)_ij / D ,  sigma = sqrt(E2 - mu^2 + eps)
  att_i  = g * ( ua_i * (beta @ UA)_i - sum_j beta_ij*mu_ij ) + b
  with beta = alpha / sigma  (uses sum_j alpha_ij = 1)
so the [N,N,D] tensor collapses into three N*N x D matmuls.

att_W / att_a fold into three D-vectors computed on host:
  s_q_i = ua_i . vq (+cq),  vq = aq @ Wa  etc.; all constants summed into C0.
"""

import numpy as np

import concourse.bass as bass
from concourse import bacc
import concourse.mybir as mybir
import concourse.tile as tile
from concourse import masks
from concourse.bass_utils import run_bass_kernel_spmd

F32 = mybir.dt.float32
ALU = mybir.AluOpType
ACTF = mybir.ActivationFunctionType

B, NODE, D = 16, 258, 64
N = NODE - 2          # 256 attention items
N_CORES = 8
B_LOC = B // N_CORES  # 2 examples per core
EPS = 1e-5
SLOPE = 0.01
OUT_ROWS = N + 1      # 257


def _ln(nc, pool, t, P, g_bc, b_bc, eps_col, pfx):
    """Emit LayerNorm of SBUF tile t [P, D] -> new tile. g_bc/b_bc are [128, D]."""
    sm = pool.tile([P, 1], F32, tag=pfx + "_sm")
    nc.vector.reduce_sum(sm[:], t[:], axis=mybir.AxisListType.X)
    nm = pool.tile([P, 1], F32, tag=pfx + "_nm")
    nc.scalar.mul(nm[:], sm[:], -1.0 / D)                    # -mean
    xc = pool.tile([P, D], F32, tag=pfx + "_xc")
    nc.vector.tensor_scalar_add(xc[:], t[:], nm[:])          # x - mean
    sq = pool.tile([P, D], F32, tag=pfx + "_sq")
    ss = pool.tile([P, 1], F32, tag=pfx + "_ss")
    nc.scalar.activation(sq[:], xc[:], ACTF.Square, accum_out=ss[:])
    std = pool.tile([P, 1], F32, tag=pfx + "_std")
    nc.scalar.activation(std[:], ss[:], ACTF.Sqrt, bias=eps_col[:P, :], scale=1.0 / D)
    rstd = pool.tile([P, 1], F32, tag=pfx + "_rstd")
    nc.vector.reciprocal(rstd[:], std[:])
    o = pool.tile([P, D], F32, tag=pfx + "_o")
    nc.vector.scalar_tensor_tensor(o[:], xc[:], rstd[:], g_bc[:P, :], op0=ALU.mult, op1=ALU.mult)
    nc.vector.tensor_add(o[:], o[:], b_bc[:P, :])
    return o


def _lrelu(nc, out_ap, in_ap):
    # leaky_relu(x) = max(0.01*x, x) in one DVE op
    nc.vector.scalar_tensor_tensor(out_ap, in_ap, SLOPE, in_ap, op0=ALU.mult, op1=ALU.max)


def build(use_act_lrelu=True):
    nc = bacc.Bacc()
    emb = nc.dram_tensor("emb", [B_LOC, NODE, D], F32, kind="ExternalInput")
    cstT = nc.dram_tensor("cstT", [D, 2], F32, kind="ExternalInput")   # cols: vq, vk
    cstR = nc.dram_tensor("cstR", [4, D], F32, kind="ExternalInput")   # rows: vi, g, b, [C0,..]
    out = nc.dram_tensor("out", [B_LOC, OUT_ROWS, D], F32, kind="ExternalOutput")

    with tile.TileContext(nc) as tc:
        with (
            tc.tile_pool(name="const", bufs=1) as cpool,
            tc.tile_pool(name="work", bufs=2) as pool,
            tc.tile_pool(name="psmall", bufs=1, space="PSUM") as psmall,
            tc.tile_pool(name="pqk", bufs=1, space="PSUM") as pqk,
            tc.tile_pool(name="pmue2", bufs=2, space="PSUM") as pmue2,
            tc.tile_pool(name="pbt", bufs=2, space="PSUM") as pbt,
            tc.tile_pool(name="ps", bufs=2, space="PSUM") as ps,
        ):
            # ---- global constants ----
            ident = cpool.tile([128, 128], F32)
            masks.make_identity(nc, ident[:])
            ones_r = cpool.tile([1, 128], F32)
            nc.vector.memset(ones_r[:], 1.0)
            eps_col = cpool.tile([128, 1], F32)
            nc.vector.memset(eps_col[:], EPS)

            vq_col = cpool.tile([D, 1], F32)
            nc.sync.dma_start(vq_col[:], cstT[:, 0:1])
            vk_col = cpool.tile([D, 1], F32)
            nc.sync.dma_start(vk_col[:], cstT[:, 1:2])
            vi_row = cpool.tile([1, D], F32)
            nc.sync.dma_start(vi_row[:], cstR[0:1, :])
            gb_row = cpool.tile([1, 2 * D], F32)
            nc.sync.dma_start(gb_row[:], cstR[1:3, :])
            c0_sb = cpool.tile([1, 1], F32)
            nc.sync.dma_start(c0_sb[:], cstR[3:4, 0:1])

            # broadcast [g|b] across 128 partitions via ones-matmul
            p_gb = psmall.tile([128, 2 * D], F32, tag="small")
            nc.tensor.matmul(p_gb[:], ones_r[:], gb_row[:])
            gb_bc = cpool.tile([128, 2 * D], F32)
            nc.scalar.copy(gb_bc[:], p_gb[:])
            g_bc = gb_bc[:, 0:D]
            b_bc = gb_bc[:, D:2 * D]

            for e in range(B_LOC):
                # ---- load + LayerNorm ----
                tA = pool.tile([128, D], F32, tag="tA")
                nc.sync.dma_start(tA[:], emb[e, 2:130, :])
                tB = pool.tile([128, D], F32, tag="tB")
                nc.sync.dma_start(tB[:], emb[e, 130:258, :])
                tU = pool.tile([2, D], F32, tag="tU")
                nc.sync.dma_start(tU[:], emb[e, 0:2, :])

                elnA = _ln(nc, pool, tA, 128, g_bc, b_bc, eps_col, "lnA")
                elnB = _ln(nc, pool, tB, 128, g_bc, b_bc, eps_col, "lnB")
                elnU = _ln(nc, pool, tU, 2, g_bc, b_bc, eps_col, "lnU")

                # iid0 row moved to partition 0 (compute reads need aligned start)
                iid_row = pool.tile([1, D], F32, tag="iid")
                nc.sync.dma_start(iid_row[:], elnU[1:2, :])

                # ---- ua = uid0 * iatt ----
                p_u0 = psmall.tile([128, D], F32, tag="small")
                nc.tensor.matmul(p_u0[:], ones_r[:], elnU[0:1, :])  # broadcast uid0
                uaA = pool.tile([128, D], F32, tag="uaA")
                nc.vector.tensor_mul(uaA[:], elnA[:], p_u0[:])
                uaB = pool.tile([128, D], F32, tag="uaB")
                nc.vector.tensor_mul(uaB[:], elnB[:], p_u0[:])

                # ---- UA^T [D, N] and (UA^2)^T ----
                p_t = psmall.tile([D, N], F32, tag="small")
                nc.tensor.transpose(p_t[:, 0:128], uaA[:], ident[:])
                nc.tensor.transpose(p_t[:, 128:256], uaB[:], ident[:])
                uat = pool.tile([D, N], F32, tag="uat")
                nc.scalar.copy(uat[:], p_t[:])
                ua2t = pool.tile([D, N], F32, tag="ua2t")
                nc.scalar.activation(ua2t[:], uat[:], ACTF.Square)

                # ---- attention scores ----
                p_sq = psmall.tile([128, 2], F32, tag="small")
                nc.tensor.matmul(p_sq[:, 0:1], uat[:, 0:128], vq_col[:])
                nc.tensor.matmul(p_sq[:, 1:2], uat[:, 128:256], vq_col[:])
                sq_sb = pool.tile([128, 2], F32, tag="sq")
                nc.scalar.copy(sq_sb[:], p_sq[:])

                p_sk = psmall.tile([1, N], F32, tag="small")
                nc.tensor.matmul(p_sk[:], vk_col[:], uat[:])

                si_scr = pool.tile([1, D], F32, tag="si_scr")
                si = pool.tile([1, 1], F32, tag="si")
                nc.vector.tensor_mul(si_scr[:], iid_row[:], vi_row[:])
                nc.vector.reduce_sum(si[:], si_scr[:], axis=mybir.AxisListType.X)
                c_all = pool.tile([1, 1], F32, tag="c_all")
                nc.vector.tensor_scalar_add(c_all[:], si[:], c0_sb[:])
                skc = pool.tile([1, N], F32, tag="skc")
                nc.vector.tensor_scalar_add(skc[:], p_sk[:], c_all[:])

                p_qk = pqk.tile([128, N], F32, tag="qk")
                nc.tensor.matmul(p_qk[:], ones_r[:], skc[:])   # bcast s_k+c over i

                betas, c_cols = [], []
                for blk in range(2):
                    cs = slice(blk * 128, (blk + 1) * 128)
                    p_mu = pmue2.tile([128, N], F32, tag="mue2")
                    nc.tensor.matmul(p_mu[:], uat[:, cs], uat[:])
                    p_e2 = pmue2.tile([128, N], F32, tag="mue2")
                    nc.tensor.matmul(p_e2[:], ua2t[:, cs], ua2t[:])

                    # qk = LeakyReLU(bcast(s_k+c) + s_q_i)
                    qk = pool.tile([128, N], F32, tag="qk_sb")
                    if use_act_lrelu:
                        nc.scalar.activation(qk[:], p_qk[:], ACTF.Lrelu,
                                             bias=sq_sb[:, blk:blk + 1], alpha=SLOPE)
                    else:
                        nc.vector.tensor_scalar_add(qk[:], p_qk[:], sq_sb[:, blk:blk + 1])
                        _lrelu(nc, qk[:], qk[:])
                    negmax = pool.tile([128, 1], F32, tag="negmax")
                    nc.vector.tensor_reduce(negmax[:], qk[:], axis=mybir.AxisListType.X,
                                            op=ALU.max, negate=True)
                    expv = pool.tile([128, N], F32, tag="expv")
                    denom = pool.tile([128, 1], F32, tag="denom")
                    nc.scalar.activation(expv[:], qk[:], ACTF.Exp,
                                         bias=negmax[:], accum_out=denom[:])
                    rden = pool.tile([128, 1], F32, tag="rden")
                    nc.vector.reciprocal(rden[:], denom[:])

                    mu_s = pool.tile([128, N], F32, tag="mu_s")
                    nc.scalar.mul(mu_s[:], p_mu[:], 1.0 / D)
                    msq = pool.tile([128, N], F32, tag="msq")
                    nc.scalar.activation(msq[:], mu_s[:], ACTF.Square)
                    var = pool.tile([128, N], F32, tag="var")
                    nc.vector.scalar_tensor_tensor(var[:], p_e2[:], 1.0 / D, msq[:],
                                                   op0=ALU.mult, op1=ALU.subtract)
                    stdt = pool.tile([128, N], F32, tag="stdt")
                    nc.scalar.activation(stdt[:], var[:], ACTF.Sqrt, bias=eps_col[:])
                    rstd = pool.tile([128, N], F32, tag="rstd")
                    nc.vector.reciprocal(rstd[:], stdt[:])

                    beta = pool.tile([128, N], F32, tag=f"beta{blk}")
                    nc.vector.scalar_tensor_tensor(beta[:], expv[:], rden[:], rstd[:],
                                                   op0=ALU.mult, op1=ALU.mult)
                    bm_scr = pool.tile([128, N], F32, tag="bm_scr")
                    c_col = pool.tile([128, 1], F32, tag=f"ccol{blk}")
                    nc.vector.tensor_mul(bm_scr[:], beta[:], mu_s[:])
                    nc.vector.reduce_sum(c_col[:], bm_scr[:], axis=mybir.AxisListType.X)
                    betas.append(beta)
                    c_cols.append(c_col)

                # ---- transpose beta (4x 128x128 on PE) ----
                p_bt0 = pbt.tile([128, N], F32, tag="bt")
                nc.tensor.transpose(p_bt0[:, 0:128], betas[0][:, 0:128], ident[:])
                nc.tensor.transpose(p_bt0[:, 128:256], betas[1][:, 0:128], ident[:])
                p_bt1 = pbt.tile([128, N], F32, tag="bt")
                nc.tensor.transpose(p_bt1[:, 0:128], betas[0][:, 128:256], ident[:])
                nc.tensor.transpose(p_bt1[:, 128:256], betas[1][:, 128:256], ident[:])
                bt0 = pool.tile([128, N], F32, tag="bt0")
                nc.scalar.copy(bt0[:], p_bt0[:])
                bt1 = pool.tile([128, N], F32, tag="bt1")
                nc.scalar.copy(bt1[:], p_bt1[:])

                # ---- S = beta @ UA ; out = LR(g*(ua*S - c) + b) ----
                for blk in range(2):
                    cs = slice(blk * 128, (blk + 1) * 128)
                    p_S = ps.tile([128, D], F32, tag="S")
                    nc.tensor.matmul(p_S[:], bt0[:, cs], uaA[:], start=True, stop=False)
                    nc.tensor.matmul(p_S[:], bt1[:, cs], uaB[:], start=False, stop=True)
                    ua_blk = uaA if blk == 0 else uaB
                    t1 = pool.tile([128, D], F32, tag="t1")
                    nc.vector.tensor_mul(t1[:], ua_blk[:], p_S[:])
                    t2 = pool.tile([128, D], F32, tag="t2")
                    nc.vector.scalar_tensor_tensor(t2[:], t1[:], c_cols[blk][:], g_bc[:, :],
                                                   op0=ALU.subtract, op1=ALU.mult)
                    t3 = pool.tile([128, D], F32, tag="t3")
                    nc.vector.tensor_add(t3[:], t2[:], b_bc[:, :])
                    o = pool.tile([128, D], F32, tag="o")
                    _lrelu(nc, o[:], t3[:])
                    nc.sync.dma_start(out[e, 1 + blk * 128:1 + (blk + 1) * 128, :], o[:])

                # ---- row 0: LR(uid0 * iid0) ----
                ui = pool.tile([1, D], F32, tag="ui")
                nc.vector.tensor_mul(ui[:], elnU[0:1, :], iid_row[:])
                uo = pool.tile([1, D], F32, tag="uo")
                _lrelu(nc, uo[:], ui[:])
                nc.sync.dma_start(out[e, 0:1, :], uo[:])

    nc.compile()
    return nc


def _host_consts(Wa, ba, a_w, a_b):
    aq, ak, ai = a_w[:D], a_w[D:2 * D], a_w[2 * D:]
    vq = aq @ Wa          # vq[d] = sum_o aq[o] * Wa[o, d]
    vk = ak @ Wa
    vi = ai @ Wa
    c0 = float(ba @ aq + ba @ ak + ba @ ai + a_b[0])
    cstT = np.stack([vq, vk], axis=1).astype(np.float32)       # [D, 2]
    cstR = np.zeros((4, D), np.float32)
    cstR[0] = vi
    cstR[1] = 0.0  # filled by caller with ln_g
    cstR[2] = 0.0  # ln_b
    cstR[3, 0] = c0
    return cstT, cstR


_NC_CACHE = {}


def _get_nc():
    if "nc" not in _NC_CACHE:
        _NC_CACHE["nc"] = build()
    return _NC_CACHE["nc"]


def run(embeddings, Wa, ba, a_w, a_b, ln_g, ln_b, **spmd_kwargs):
    embeddings = np.ascontiguousarray(embeddings, dtype=np.float32)
    cstT, cstR = _host_consts(np.asarray(Wa, np.float32), np.asarray(ba, np.float32),
                              np.asarray(a_w, np.float32), np.asarray(a_b, np.float32))
    cstR[1] = np.asarray(ln_g, np.float32)
    cstR[2] = np.asarray(ln_b, np.float32)

    nc = _get_nc()
    in_maps = [
        {"emb": embeddings[c * B_LOC:(c + 1) * B_LOC], "cstT": cstT, "cstR": cstR}
        for c in range(N_CORES)
    ]
    res = run_bass_kernel_spmd(nc, in_maps, core_ids=list(range(N_CORES)), **spmd_kwargs)
    outp = np.concatenate([res.results[c]["out"] for c in range(N_CORES)], axis=0)
    return outp, res


def kernel(embeddings, Wa, ba, a_w, a_b, ln_g, ln_b):
    outp, _ = run(embeddings, Wa, ba, a_w, a_b, ln_g, ln_b)
    return outp


# revision 10
# speedup vs baseline: 1.3976x; 1.3976x over previous
"""GAT attention kernel for Trainium2 (Bass/Tile), 8-core data parallel.

Per-core math (2 examples each, N=256 items, D=64):
  e   = LayerNorm(emb);  ua = e[0] * e[2:]
  qk  = LeakyReLU(s_q_i + s_k_j + c);  alpha = softmax_j
  attention over value_ij = LN(ua_i * ua_j) collapsed via gram matrices:
    mu = UA@UA^T/D,  E2 = UA^2@UA^2^T/D,  invs = rsqrt(E2 - mu^2)
    att_i = g*(ua_i*(beta@UA)_i - sum_j beta_ij mu_ij) + b,  beta = alpha*invs
  out = LeakyReLU(concat([e0*e1], att))

Perf notes:
  - ALL ACT funcs from one table set (Prelu/Exp/Copy/Square/Identity):
    zero act-table reloads. LeakyReLU = Prelu(alpha=0.01) with fused bias.
  - rsqrt on DVE via quake bit-trick + 1 Newton step (no Sqrt table,
    no slow InstReciprocal on the [128,256] tiles).
  - all matmuls in fp32r (1-pass PE instead of fp32's 4-pass); producers
    write fp32r-rounded tiles.
"""

import numpy as np

import concourse.bass as bass
from concourse import bacc
import concourse.mybir as mybir
import concourse.tile as tile
from concourse import masks
from concourse.bass_utils import run_bass_kernel_spmd

F32 = mybir.dt.float32
F32R = mybir.dt.float32r
I32 = mybir.dt.int32
ALU = mybir.AluOpType
ACTF = mybir.ActivationFunctionType
AX = mybir.AxisListType

B, NODE, D = 16, 258, 64
N = NODE - 2
N_CORES = 8
B_LOC = B // N_CORES
EPS = 1e-5
SLOPE = 0.01
OUT_ROWS = N + 1
MAGIC = 0x5f375a86


def _rsqrt(nc, pool, x, P, W, pfx):
    """rstd = x**-0.5 on DVE: bit trick + 1 Newton iteration (6 ops).

    max rel err ~1.8e-3; x must be > 0.
    """
    y0 = pool.tile([P, W], F32, tag=pfx + "_y0")
    nc.vector.tensor_scalar(y0.bitcast(I32)[:], x.bitcast(I32)[:], 1, None,
                            op0=ALU.logical_shift_right)
    nc.vector.tensor_scalar(y0.bitcast(I32)[:], y0.bitcast(I32)[:], -1, MAGIC,
                            op0=ALU.mult, op1=ALU.add)
    t = pool.tile([P, W], F32, tag=pfx + "_t")
    nc.vector.tensor_mul(t[:], y0[:], y0[:])
    u = pool.tile([P, W], F32, tag=pfx + "_u")
    nc.vector.scalar_tensor_tensor(u[:], t[:], 0.5, x[:], op0=ALU.mult, op1=ALU.mult)
    v = pool.tile([P, W], F32, tag=pfx + "_v")
    nc.vector.tensor_mul(v[:], u[:], y0[:])
    r = pool.tile([P, W], F32, tag=pfx + "_r")
    nc.vector.scalar_tensor_tensor(r[:], y0[:], 1.5, v[:], op0=ALU.mult, op1=ALU.subtract)
    return r


def _lrelu(nc, out_ap, in_ap):
    nc.vector.scalar_tensor_tensor(out_ap, in_ap, SLOPE, in_ap, op0=ALU.mult, op1=ALU.max)


def build():
    nc = bacc.Bacc()
    emb = nc.dram_tensor("emb", [B_LOC, NODE, D], F32, kind="ExternalInput")
    cstT = nc.dram_tensor("cstT", [D, 2], F32, kind="ExternalInput")   # cols: vq, vk
    cstR = nc.dram_tensor("cstR", [4, D], F32, kind="ExternalInput")   # vi, g, b, [C0]
    out = nc.dram_tensor("out", [B_LOC, OUT_ROWS, D], F32, kind="ExternalOutput")

    with tile.TileContext(nc) as tc:
        with (
            tc.tile_pool(name="const", bufs=1) as cpool,
            tc.tile_pool(name="work", bufs=2) as pool,
            tc.tile_pool(name="psmall", bufs=1, space="PSUM") as psmall,
            tc.tile_pool(name="pqk", bufs=1, space="PSUM") as pqk,
            tc.tile_pool(name="pmue2", bufs=2, space="PSUM") as pmue2,
            tc.tile_pool(name="pbt", bufs=2, space="PSUM") as pbt,
            tc.tile_pool(name="ps", bufs=2, space="PSUM") as ps,
        ):
            # ---- global constants ----
            identF = cpool.tile([128, 128], F32)
            masks.make_identity(nc, identF[:])
            identR = cpool.tile([128, 128], F32R)
            nc.scalar.copy(identR[:], identF[:])
            ones_f = cpool.tile([1, 128], F32)
            nc.vector.memset(ones_f[:], 1.0)
            ones_r = cpool.tile([1, 128], F32R)
            nc.scalar.copy(ones_r[:], ones_f[:])

            vqk = cpool.tile([D, 2], F32)
            nc.sync.dma_start(vqk[:], cstT[:, :])
            vqkr = cpool.tile([D, 2], F32R)
            nc.scalar.copy(vqkr[:], vqk[:])
            vi_row = cpool.tile([1, D], F32)
            nc.sync.dma_start(vi_row[:], cstR[0:1, :])
            gb_row = cpool.tile([1, 2 * D], F32)
            nc.sync.dma_start(gb_row[:], cstR[1:3, :])
            gb_rowr = cpool.tile([1, 2 * D], F32R)
            nc.scalar.copy(gb_rowr[:], gb_row[:])
            c0_sb = cpool.tile([1, 1], F32)
            nc.sync.dma_start(c0_sb[:], cstR[3:4, 0:1])

            p_gb = psmall.tile([128, 2 * D], F32, tag="small")
            nc.tensor.matmul(p_gb[:], ones_r[:], gb_rowr[:])
            gb_bc = cpool.tile([128, 2 * D], F32)
            nc.scalar.copy(gb_bc[:], p_gb[:])
            g_bc = gb_bc[:, 0:D]
            b_bc = gb_bc[:, D:2 * D]

            for e in range(B_LOC):
                # ---- load ----
                tA = pool.tile([128, D], F32, tag="tA")
                nc.sync.dma_start(tA[:], emb[e, 2:130, :])
                tB = pool.tile([128, D], F32, tag="tB")
                nc.sync.dma_start(tB[:], emb[e, 130:258, :])
                tU = pool.tile([2, D], F32, tag="tU")
                nc.sync.dma_start(tU[:], emb[e, 0:2, :])

                # ---- batched LayerNorm (A, B, U share one rsqrt chain) ----
                sum_b = pool.tile([128, 3], F32, tag="sum_b")
                nc.vector.reduce_sum(sum_b[:, 0:1], tA[:], axis=AX.X)
                nc.vector.reduce_sum(sum_b[:, 1:2], tB[:], axis=AX.X)
                nc.vector.reduce_sum(sum_b[0:2, 2:3], tU[:], axis=AX.X)
                nm_b = pool.tile([128, 3], F32, tag="nm_b")
                nc.scalar.mul(nm_b[:], sum_b[:], -1.0 / D)

                xcA = pool.tile([128, D], F32, tag="xcA")
                nc.vector.tensor_scalar_add(xcA[:], tA[:], nm_b[:, 0:1])
                xcB = pool.tile([128, D], F32, tag="xcB")
                nc.vector.tensor_scalar_add(xcB[:], tB[:], nm_b[:, 1:2])
                xcU = pool.tile([2, D], F32, tag="xcU")
                nc.vector.tensor_scalar_add(xcU[:], tU[:], nm_b[0:2, 2:3])

                ss_b = pool.tile([128, 3], F32, tag="ss_b")
                sqs = pool.tile([128, D], F32, tag="sqs")
                nc.scalar.activation(sqs[:], xcA[:], ACTF.Square, accum_out=ss_b[:, 0:1])
                sqs2 = pool.tile([128, D], F32, tag="sqs2")
                nc.scalar.activation(sqs2[:], xcB[:], ACTF.Square, accum_out=ss_b[:, 1:2])
                squ = pool.tile([2, D], F32, tag="squ")
                nc.scalar.activation(squ[:], xcU[:], ACTF.Square, accum_out=ss_b[0:2, 2:3])

                xv = pool.tile([128, 3], F32, tag="xv")
                nc.vector.tensor_scalar(xv[:], ss_b[:], 1.0 / D, EPS, op0=ALU.mult, op1=ALU.add)
                rstd_b = _rsqrt(nc, pool, xv, 128, 3, "lnr")

                elnA = pool.tile([128, D], F32, tag="elnA")
                nc.vector.scalar_tensor_tensor(elnA[:], xcA[:], rstd_b[:, 0:1], g_bc,
                                               op0=ALU.mult, op1=ALU.mult)
                nc.vector.tensor_add(elnA[:], elnA[:], b_bc)
                elnB = pool.tile([128, D], F32, tag="elnB")
                nc.vector.scalar_tensor_tensor(elnB[:], xcB[:], rstd_b[:, 1:2], g_bc,
                                               op0=ALU.mult, op1=ALU.mult)
                nc.vector.tensor_add(elnB[:], elnB[:], b_bc)
                elnU = pool.tile([2, D], F32R, tag="elnU")
                nc.vector.scalar_tensor_tensor(elnU[:], xcU[:], rstd_b[0:2, 2:3],
                                               g_bc[0:2, :], op0=ALU.mult, op1=ALU.mult)
                nc.vector.tensor_add(elnU[:], elnU.bitcast(F32)[:], b_bc[0:2, :])

                iid_row = pool.tile([1, D], F32, tag="iid")
                nc.sync.dma_start(iid_row[:], elnU.bitcast(F32)[1:2, :])

                # ---- ua = uid0 * iatt  (fp32r for matmul inputs) ----
                p_u0 = psmall.tile([128, D], F32, tag="small")
                nc.tensor.matmul(p_u0[:], ones_r[:], elnU[0:1, :])
                uaA = pool.tile([128, D], F32R, tag="uaA")
                nc.vector.tensor_mul(uaA[:], elnA[:], p_u0[:])
                uaB = pool.tile([128, D], F32R, tag="uaB")
                nc.vector.tensor_mul(uaB[:], elnB[:], p_u0[:])

                # ---- UA^T and (UA^2)^T ----
                p_t = psmall.tile([D, N], F32R, tag="small")
                nc.tensor.transpose(p_t[:, 0:128], uaA[:], identR[:])
                nc.tensor.transpose(p_t[:, 128:256], uaB[:], identR[:])
                uat = pool.tile([D, N], F32R, tag="uat")
                nc.scalar.copy(uat[:], p_t.bitcast(F32)[:])
                ua2t = pool.tile([D, N], F32R, tag="ua2t")
                nc.scalar.activation(ua2t[:], uat.bitcast(F32)[:], ACTF.Square)

                # ---- scores (fp32r needs rhs free >= 2: compute [sq|sk] cols) ----
                sq_sb = pool.tile([128, 2], F32, tag="sq")
                p_sqk0 = psmall.tile([128, 2], F32, tag="small")
                nc.tensor.matmul(p_sqk0[:], uat[:, 0:128], vqkr[:, 0:2])
                nc.scalar.copy(sq_sb[:, 0:1], p_sqk0[:, 0:1])
                p_sqk1 = psmall.tile([128, 2], F32, tag="small")
                nc.tensor.matmul(p_sqk1[:], uat[:, 128:256], vqkr[:, 0:2])
                nc.scalar.copy(sq_sb[:, 1:2], p_sqk1[:, 0:1])

                p_sk = psmall.tile([1, N], F32, tag="small")
                nc.tensor.matmul(p_sk[:], vqkr[:, 1:2], uat[:])

                si_scr = pool.tile([1, D], F32, tag="si_scr")
                nc.vector.tensor_mul(si_scr[:], iid_row[:], vi_row[:])
                si = pool.tile([1, 1], F32, tag="si")
                nc.vector.reduce_sum(si[:], si_scr[:], axis=AX.X)
                c_all = pool.tile([1, 1], F32, tag="c_all")
                nc.vector.tensor_scalar_add(c_all[:], si[:], c0_sb[:])
                skc = pool.tile([1, N], F32R, tag="skc")
                nc.vector.tensor_scalar_add(skc[:], p_sk[:], c_all[:])

                p_qk = pqk.tile([128, N], F32, tag="qk")
                nc.tensor.matmul(p_qk[:], ones_r[:], skc[:])

                betas, c_cols = [], []
                for blk in range(2):
                    cs = slice(blk * 128, (blk + 1) * 128)
                    p_mu = pmue2.tile([128, N], F32, tag="mue2")
                    nc.tensor.matmul(p_mu[:], uat[:, cs], uat[:])
                    p_e2 = pmue2.tile([128, N], F32, tag="mue2")
                    nc.tensor.matmul(p_e2[:], ua2t[:, cs], ua2t[:])

                    # qk = LeakyReLU(bcast(s_k+c) + s_q_i): Prelu w/ fused bias
                    qk = pool.tile([128, N], F32, tag="qk_sb")
                    nc.scalar.activation(qk[:], p_qk[:], ACTF.Prelu,
                                         bias=sq_sb[:, blk:blk + 1], alpha=SLOPE)
                    expv = pool.tile([128, N], F32, tag="expv")
                    denom = pool.tile([128, 1], F32, tag="denom")
                    nc.scalar.activation(expv[:], qk[:], ACTF.Exp, accum_out=denom[:])
                    rden = pool.tile([128, 1], F32, tag="rden")
                    nc.vector.reciprocal(rden[:], denom[:])

                    mu_s = pool.tile([128, N], F32, tag="mu_s")
                    nc.scalar.mul(mu_s[:], p_mu[:], 1.0 / D)
                    msq = pool.tile([128, N], F32, tag="msq")
                    nc.scalar.activation(msq[:], mu_s[:], ACTF.Square)
                    var = pool.tile([128, N], F32, tag="var")
                    nc.vector.scalar_tensor_tensor(var[:], p_e2[:], 1.0 / D, msq[:],
                                                   op0=ALU.mult, op1=ALU.subtract)
                    rstd = _rsqrt(nc, pool, var, 128, N, f"rs{blk}")

                    beta = pool.tile([128, N], F32R, tag=f"beta{blk}")
                    nc.vector.scalar_tensor_tensor(beta[:], expv[:], rden[:],
                                                   rstd[:], op0=ALU.mult, op1=ALU.mult)
                    bm_scr = pool.tile([128, N], F32, tag="bm_scr")
                    nc.vector.tensor_mul(bm_scr[:], beta.bitcast(F32)[:], mu_s[:])
                    c_col = pool.tile([128, 1], F32, tag=f"ccol{blk}")
                    nc.vector.reduce_sum(c_col[:], bm_scr[:], axis=AX.X)
                    betas.append(beta)
                    c_cols.append(c_col)

                # ---- transpose beta ----
                p_bt0 = pbt.tile([128, N], F32R, tag="bt")
                nc.tensor.transpose(p_bt0[:, 0:128], betas[0][:, 0:128], identR[:])
                nc.tensor.transpose(p_bt0[:, 128:256], betas[1][:, 0:128], identR[:])
                p_bt1 = pbt.tile([128, N], F32R, tag="bt")
                nc.tensor.transpose(p_bt1[:, 0:128], betas[0][:, 128:256], identR[:])
                nc.tensor.transpose(p_bt1[:, 128:256], betas[1][:, 128:256], identR[:])
                bt0 = pool.tile([128, N], F32R, tag="bt0")
                nc.scalar.copy(bt0[:], p_bt0.bitcast(F32)[:])
                bt1 = pool.tile([128, N], F32R, tag="bt1")
                nc.scalar.copy(bt1[:], p_bt1.bitcast(F32)[:])

                # ---- S = beta @ UA ; out rows ----
                for blk in range(2):
                    cs = slice(blk * 128, (blk + 1) * 128)
                    p_S = ps.tile([128, D], F32, tag="S")
                    nc.tensor.matmul(p_S[:], bt0[:, cs], uaA[:], start=True, stop=False)
                    nc.tensor.matmul(p_S[:], bt1[:, cs], uaB[:], start=False, stop=True)
                    ua_blk = uaA if blk == 0 else uaB
                    t1 = pool.tile([128, D], F32, tag="t1")
                    nc.vector.tensor_mul(t1[:], ua_blk.bitcast(F32)[:], p_S[:])
                    t2 = pool.tile([128, D], F32, tag="t2")
                    nc.vector.scalar_tensor_tensor(t2[:], t1[:], c_cols[blk][:], g_bc,
                                                   op0=ALU.subtract, op1=ALU.mult)
                    t3 = pool.tile([128, D], F32, tag="t3")
                    nc.vector.tensor_add(t3[:], t2[:], b_bc)
                    o = pool.tile([128, D], F32, tag="o")
                    _lrelu(nc, o[:], t3[:])
                    nc.sync.dma_start(out[e, 1 + blk * 128:1 + (blk + 1) * 128, :], o[:])

                # ---- row 0 ----
                ui = pool.tile([1, D], F32, tag="ui")
                nc.vector.tensor_mul(ui[:], elnU.bitcast(F32)[0:1, :], iid_row[:])
                uo = pool.tile([1, D], F32, tag="uo")
                _lrelu(nc, uo[:], ui[:])
                nc.sync.dma_start(out[e, 0:1, :], uo[:])

    nc.compile()
    return nc


def _host_consts(Wa, ba, a_w, a_b):
    aq, ak, ai = a_w[:D], a_w[D:2 * D], a_w[2 * D:]
    vq = aq @ Wa
    vk = ak @ Wa
    vi = ai @ Wa
    c0 = float(ba @ aq + ba @ ak + ba @ ai + a_b[0])
    cstT = np.stack([vq, vk], axis=1).astype(np.float32)
    cstR = np.zeros((4, D), np.float32)
    cstR[0] = vi
    cstR[3, 0] = c0
    return cstT, cstR


_NC_CACHE = {}


def _get_nc():
    if "nc" not in _NC_CACHE:
        _NC_CACHE["nc"] = build()
    return _NC_CACHE["nc"]


def run(embeddings, Wa, ba, a_w, a_b, ln_g, ln_b, **spmd_kwargs):
    embeddings = np.ascontiguousarray(embeddings, dtype=np.float32)
    cstT, cstR = _host_consts(np.asarray(Wa, np.float32), np.asarray(ba, np.float32),
                              np.asarray(a_w, np.float32), np.asarray(a_b, np.float32))
    cstR[1] = np.asarray(ln_g, np.float32)
    cstR[2] = np.asarray(ln_b, np.float32)

    nc = _get_nc()
    in_maps = [
        {"emb": embeddings[c * B_LOC:(c + 1) * B_LOC], "cstT": cstT, "cstR": cstR}
        for c in range(N_CORES)
    ]
    res = run_bass_kernel_spmd(nc, in_maps, core_ids=list(range(N_CORES)), **spmd_kwargs)
    outp = np.concatenate([res.results[c]["out"] for c in range(N_CORES)], axis=0)
    return outp, res


def kernel(embeddings, Wa, ba, a_w, a_b, ln_g, ln_b):
    outp, _ = run(embeddings, Wa, ba, a_w, a_b, ln_g, ln_b)
    return outp


# revision 12
# speedup vs baseline: 1.4424x; 1.0321x over previous
"""GAT attention kernel for Trainium2 (Bass/Tile), 8-core data parallel.

Per-core math (2 examples each, N=256 items, D=64):
  e   = LayerNorm(emb);  ua = e[0] * e[2:]
  qk  = LeakyReLU(s_q_i + s_k_j + c);  alpha = softmax_j
  attention over value_ij = LN(ua_i * ua_j) collapsed via gram matrices:
    mu = UA@UA^T/D,  E2 = UA^2@UA^2^T/D,  invs = rsqrt(E2 - mu^2)
    att_i = g*(ua_i*(beta@UA)_i - sum_j beta_ij mu_ij) + b,  beta = alpha*invs
  out = LeakyReLU(concat([e0*e1], att))

Perf notes:
  - ALL ACT funcs from one table set (Prelu/Exp/Copy/Square/Identity):
    zero act-table reloads. LeakyReLU = Prelu(alpha=0.01) with fused bias.
  - rsqrt on DVE via quake bit-trick + 1 Newton step (no Sqrt table,
    no slow InstReciprocal on the [128,256] tiles).
  - all matmuls in fp32r (1-pass PE instead of fp32's 4-pass); producers
    write fp32r-rounded tiles.
"""

import numpy as np

import concourse.bass as bass
from concourse import bacc
import concourse.mybir as mybir
import concourse.tile as tile
from concourse import masks
from concourse.bass_utils import run_bass_kernel_spmd

F32 = mybir.dt.float32
F32R = mybir.dt.float32r
I32 = mybir.dt.int32
ALU = mybir.AluOpType
ACTF = mybir.ActivationFunctionType
AX = mybir.AxisListType

B, NODE, D = 16, 258, 64
N = NODE - 2
N_CORES = 8
B_LOC = B // N_CORES
EPS = 1e-5
SLOPE = 0.01
OUT_ROWS = N + 1
MAGIC = 0x5f375a86


def _rsqrt(nc, pool, x, P, W, pfx):
    """rstd = x**-0.5 on DVE: bit trick + 1 Newton iteration (6 ops).

    max rel err ~1.8e-3; x must be > 0.
    """
    y0 = pool.tile([P, W], F32, tag=pfx + "_y0")
    nc.vector.tensor_scalar(y0.bitcast(I32)[:], x.bitcast(I32)[:], 1, None,
                            op0=ALU.logical_shift_right)
    nc.vector.tensor_scalar(y0.bitcast(I32)[:], y0.bitcast(I32)[:], -1, MAGIC,
                            op0=ALU.mult, op1=ALU.add)
    t = pool.tile([P, W], F32, tag=pfx + "_t")
    nc.vector.tensor_mul(t[:], y0[:], y0[:])
    u = pool.tile([P, W], F32, tag=pfx + "_u")
    nc.vector.scalar_tensor_tensor(u[:], t[:], 0.5, x[:], op0=ALU.mult, op1=ALU.mult)
    v = pool.tile([P, W], F32, tag=pfx + "_v")
    nc.vector.tensor_mul(v[:], u[:], y0[:])
    r = pool.tile([P, W], F32, tag=pfx + "_r")
    nc.vector.scalar_tensor_tensor(r[:], y0[:], 1.5, v[:], op0=ALU.mult, op1=ALU.subtract)
    return r


def _lrelu(nc, out_ap, in_ap):
    nc.vector.scalar_tensor_tensor(out_ap, in_ap, SLOPE, in_ap, op0=ALU.mult, op1=ALU.max)


def build():
    nc = bacc.Bacc()
    emb = nc.dram_tensor("emb", [B_LOC, NODE, D], F32, kind="ExternalInput")
    cstT = nc.dram_tensor("cstT", [D, 2], F32, kind="ExternalInput")   # cols: vq, vk
    cstR = nc.dram_tensor("cstR", [4, D], F32, kind="ExternalInput")   # vi, g, b, [C0]
    out = nc.dram_tensor("out", [B_LOC, OUT_ROWS, D], F32, kind="ExternalOutput")

    with tile.TileContext(nc) as tc:
        with (
            tc.tile_pool(name="const", bufs=1) as cpool,
            tc.tile_pool(name="work", bufs=2) as pool,
            tc.tile_pool(name="psmall", bufs=1, space="PSUM") as psmall,
            tc.tile_pool(name="pqk", bufs=1, space="PSUM") as pqk,
            tc.tile_pool(name="pmue2", bufs=2, space="PSUM") as pmue2,
            tc.tile_pool(name="pbt", bufs=2, space="PSUM") as pbt,
            tc.tile_pool(name="ps", bufs=2, space="PSUM") as ps,
        ):
            # ---- global constants ----
            identF = cpool.tile([128, 128], F32)
            masks.make_identity(nc, identF[:])
            identR = cpool.tile([128, 128], F32R)
            nc.scalar.copy(identR[:], identF[:])
            ones_f = cpool.tile([1, 128], F32)
            nc.vector.memset(ones_f[:], 1.0)
            ones_r = cpool.tile([1, 128], F32R)
            nc.scalar.copy(ones_r[:], ones_f[:])

            vqk = cpool.tile([D, 2], F32)
            nc.sync.dma_start(vqk[:], cstT[:, :])
            vqkr = cpool.tile([D, 2], F32R)
            nc.scalar.copy(vqkr[:], vqk[:])
            vi_row = cpool.tile([1, D], F32)
            nc.sync.dma_start(vi_row[:], cstR[0:1, :])
            gb_row = cpool.tile([1, 2 * D], F32)
            nc.sync.dma_start(gb_row[:], cstR[1:3, :])
            gb_rowr = cpool.tile([1, 2 * D], F32R)
            nc.scalar.copy(gb_rowr[:], gb_row[:])
            c0_sb = cpool.tile([1, 1], F32)
            nc.sync.dma_start(c0_sb[:], cstR[3:4, 0:1])

            p_gb = psmall.tile([128, 2 * D], F32, tag="small")
            nc.tensor.matmul(p_gb[:], ones_r[:], gb_rowr[:])
            gb_bc = cpool.tile([128, 2 * D], F32)
            nc.scalar.copy(gb_bc[:], p_gb[:])
            g_bc = gb_bc[:, 0:D]
            b_bc = gb_bc[:, D:2 * D]

            for e in range(B_LOC):
                # ---- load ----
                tA = pool.tile([128, D], F32, tag="tA")
                nc.sync.dma_start(tA[:], emb[e, 2:130, :])
                tB = pool.tile([128, D], F32, tag="tB")
                nc.sync.dma_start(tB[:], emb[e, 130:258, :])
                tU = pool.tile([2, D], F32, tag="tU")
                nc.sync.dma_start(tU[:], emb[e, 0:2, :])

                # ---- batched LayerNorm (A, B, U share one rsqrt chain) ----
                sum_b = pool.tile([128, 3], F32, tag="sum_b")
                nc.vector.reduce_sum(sum_b[:, 0:1], tA[:], axis=AX.X)
                nc.vector.reduce_sum(sum_b[:, 1:2], tB[:], axis=AX.X)
                nc.vector.reduce_sum(sum_b[0:2, 2:3], tU[:], axis=AX.X)
                nm_b = pool.tile([128, 3], F32, tag="nm_b")
                nc.scalar.mul(nm_b[:], sum_b[:], -1.0 / D)

                xcA = pool.tile([128, D], F32, tag="xcA")
                nc.vector.tensor_scalar_add(xcA[:], tA[:], nm_b[:, 0:1])
                xcB = pool.tile([128, D], F32, tag="xcB")
                nc.vector.tensor_scalar_add(xcB[:], tB[:], nm_b[:, 1:2])
                xcU = pool.tile([2, D], F32, tag="xcU")
                nc.vector.tensor_scalar_add(xcU[:], tU[:], nm_b[0:2, 2:3])

                ss_b = pool.tile([128, 3], F32, tag="ss_b")
                sqs = pool.tile([128, D], F32, tag="sqs")
                nc.scalar.activation(sqs[:], xcA[:], ACTF.Square, accum_out=ss_b[:, 0:1])
                sqs2 = pool.tile([128, D], F32, tag="sqs2")
                nc.scalar.activation(sqs2[:], xcB[:], ACTF.Square, accum_out=ss_b[:, 1:2])
                squ = pool.tile([2, D], F32, tag="squ")
                nc.scalar.activation(squ[:], xcU[:], ACTF.Square, accum_out=ss_b[0:2, 2:3])

                xv = pool.tile([128, 3], F32, tag="xv")
                nc.vector.tensor_scalar(xv[:], ss_b[:], 1.0 / D, EPS, op0=ALU.mult, op1=ALU.add)
                rstd_b = _rsqrt(nc, pool, xv, 128, 3, "lnr")

                elnA = pool.tile([128, D], F32, tag="elnA")
                nc.vector.scalar_tensor_tensor(elnA[:], xcA[:], rstd_b[:, 0:1], g_bc,
                                               op0=ALU.mult, op1=ALU.mult)
                nc.vector.tensor_add(elnA[:], elnA[:], b_bc)
                elnB = pool.tile([128, D], F32, tag="elnB")
                nc.vector.scalar_tensor_tensor(elnB[:], xcB[:], rstd_b[:, 1:2], g_bc,
                                               op0=ALU.mult, op1=ALU.mult)
                nc.vector.tensor_add(elnB[:], elnB[:], b_bc)
                elnU = pool.tile([2, D], F32R, tag="elnU")
                nc.vector.scalar_tensor_tensor(elnU[:], xcU[:], rstd_b[0:2, 2:3],
                                               g_bc[0:2, :], op0=ALU.mult, op1=ALU.mult)
                nc.vector.tensor_add(elnU[:], elnU.bitcast(F32)[:], b_bc[0:2, :])

                iid_row = pool.tile([1, D], F32, tag="iid")
                nc.sync.dma_start(iid_row[:], elnU.bitcast(F32)[1:2, :])

                # ---- ua = uid0 * iatt  (fp32r for matmul inputs) ----
                p_u0 = psmall.tile([128, D], F32, tag="small")
                nc.tensor.matmul(p_u0[:], ones_r[:], elnU[0:1, :])
                uaA = pool.tile([128, D], F32R, tag="uaA")
                nc.vector.tensor_mul(uaA[:], elnA[:], p_u0[:])
                uaB = pool.tile([128, D], F32R, tag="uaB")
                nc.vector.tensor_mul(uaB[:], elnB[:], p_u0[:])

                # ---- UA^T and (UA^2)^T ----
                p_t = psmall.tile([D, N], F32R, tag="small")
                nc.tensor.transpose(p_t[:, 0:128], uaA[:], identR[:])
                nc.tensor.transpose(p_t[:, 128:256], uaB[:], identR[:])
                uat = pool.tile([D, N], F32R, tag="uat")
                nc.scalar.copy(uat[:], p_t.bitcast(F32)[:])
                ua2t = pool.tile([D, N], F32R, tag="ua2t")
                nc.scalar.activation(ua2t[:], uat.bitcast(F32)[:], ACTF.Square)

                # ---- scores (fp32r needs rhs free >= 2: compute [sq|sk] cols) ----
                sq_sb = pool.tile([128, 2], F32, tag="sq")
                p_sqk0 = psmall.tile([128, 2], F32, tag="small")
                nc.tensor.matmul(p_sqk0[:], uat[:, 0:128], vqkr[:, 0:2])
                nc.scalar.copy(sq_sb[:, 0:1], p_sqk0[:, 0:1])
                p_sqk1 = psmall.tile([128, 2], F32, tag="small")
                nc.tensor.matmul(p_sqk1[:], uat[:, 128:256], vqkr[:, 0:2])
                nc.scalar.copy(sq_sb[:, 1:2], p_sqk1[:, 0:1])

                p_sk = psmall.tile([1, N], F32, tag="small")
                nc.tensor.matmul(p_sk[:], vqkr[:, 1:2], uat[:])

                si_scr = pool.tile([1, D], F32, tag="si_scr")
                nc.vector.tensor_mul(si_scr[:], iid_row[:], vi_row[:])
                si = pool.tile([1, 1], F32, tag="si")
                nc.vector.reduce_sum(si[:], si_scr[:], axis=AX.X)
                c_all = pool.tile([1, 1], F32, tag="c_all")
                nc.vector.tensor_scalar_add(c_all[:], si[:], c0_sb[:])
                skc = pool.tile([1, N], F32R, tag="skc")
                nc.vector.tensor_scalar_add(skc[:], p_sk[:], c_all[:])

                p_qk = pqk.tile([128, N], F32, tag="qk")
                nc.tensor.matmul(p_qk[:], ones_r[:], skc[:])

                # batched var/rsqrt over both blocks: [128, 512]
                msq_b = pool.tile([128, 2 * N], F32, tag="msq_b")
                e2s_b = pool.tile([128, 2 * N], F32, tag="e2s_b")
                expvs, rdens = [], []
                for blk in range(2):
                    cs = slice(blk * 128, (blk + 1) * 128)
                    ns = slice(blk * N, (blk + 1) * N)
                    p_mu = pmue2.tile([128, N], F32, tag="mue2")
                    nc.tensor.matmul(p_mu[:], uat[:, cs], uat[:])
                    p_e2 = pmue2.tile([128, N], F32, tag="mue2")
                    nc.tensor.matmul(p_e2[:], ua2t[:, cs], ua2t[:])
                    nc.scalar.activation(msq_b[:, ns], p_mu[:], ACTF.Square, scale=1.0 / D)
                    nc.scalar.mul(e2s_b[:, ns], p_e2[:], 1.0 / D)

                    # qk = LeakyReLU(bcast(s_k+c) + s_q_i): Prelu w/ fused bias
                    qk = pool.tile([128, N], F32, tag="qk_sb")
                    nc.scalar.activation(qk[:], p_qk[:], ACTF.Prelu,
                                         bias=sq_sb[:, blk:blk + 1], alpha=SLOPE)
                    expv = pool.tile([128, N], F32, tag=f"expv{blk}")
                    denom = pool.tile([128, 1], F32, tag=f"denom{blk}")
                    nc.scalar.activation(expv[:], qk[:], ACTF.Exp, accum_out=denom[:])
                    rden = pool.tile([128, 1], F32, tag=f"rden{blk}")
                    nc.vector.reciprocal(rden[:], denom[:])
                    expvs.append(expv)
                    rdens.append(rden)

                var_b = pool.tile([128, 2 * N], F32, tag="var_b")
                nc.vector.tensor_sub(var_b[:], e2s_b[:], msq_b[:])
                rstd_b2 = _rsqrt(nc, pool, var_b, 128, 2 * N, "rsb")

                betas = []
                for blk in range(2):
                    ns = slice(blk * N, (blk + 1) * N)
                    beta = pool.tile([128, N], F32R, tag=f"beta{blk}")
                    nc.vector.scalar_tensor_tensor(beta[:], expvs[blk][:], rdens[blk][:],
                                                   rstd_b2[:, ns], op0=ALU.mult, op1=ALU.mult)
                    betas.append(beta)

                # ---- transpose beta ----
                p_bt0 = pbt.tile([128, N], F32R, tag="bt")
                nc.tensor.transpose(p_bt0[:, 0:128], betas[0][:, 0:128], identR[:])
                nc.tensor.transpose(p_bt0[:, 128:256], betas[1][:, 0:128], identR[:])
                p_bt1 = pbt.tile([128, N], F32R, tag="bt")
                nc.tensor.transpose(p_bt1[:, 0:128], betas[0][:, 128:256], identR[:])
                nc.tensor.transpose(p_bt1[:, 128:256], betas[1][:, 128:256], identR[:])
                bt0 = pool.tile([128, N], F32R, tag="bt0")
                nc.scalar.copy(bt0[:], p_bt0.bitcast(F32)[:])
                bt1 = pool.tile([128, N], F32R, tag="bt1")
                nc.scalar.copy(bt1[:], p_bt1.bitcast(F32)[:])

                # ---- S = beta @ UA ; c_i = rowsum(ua_i*S_i)/D ; out rows ----
                c_raw = pool.tile([128, 2], F32, tag="c_raw")
                t1s = []
                for blk in range(2):
                    cs = slice(blk * 128, (blk + 1) * 128)
                    p_S = ps.tile([128, D], F32, tag="S")
                    nc.tensor.matmul(p_S[:], bt0[:, cs], uaA[:], start=True, stop=False)
                    nc.tensor.matmul(p_S[:], bt1[:, cs], uaB[:], start=False, stop=True)
                    ua_blk = uaA if blk == 0 else uaB
                    t1 = pool.tile([128, D], F32, tag=f"t1_{blk}")
                    nc.vector.tensor_mul(t1[:], ua_blk.bitcast(F32)[:], p_S[:])
                    nc.vector.reduce_sum(c_raw[:, blk:blk + 1], t1[:], axis=AX.X)
                    t1s.append(t1)
                c_col = pool.tile([128, 2], F32, tag="c_col")
                nc.scalar.mul(c_col[:], c_raw[:], 1.0 / D)
                o_big = pool.tile([128, 2, D], F32, tag="o_big")
                for blk in range(2):
                    t2 = pool.tile([128, D], F32, tag="t2")
                    nc.vector.scalar_tensor_tensor(t2[:], t1s[blk][:], c_col[:, blk:blk + 1],
                                                   g_bc, op0=ALU.subtract, op1=ALU.mult)
                    t3 = pool.tile([128, D], F32, tag="t3")
                    nc.vector.tensor_add(t3[:], t2[:], b_bc)
                    nc.scalar.activation(o_big[:, blk, :], t3[:],
                                         ACTF.Prelu, alpha=SLOPE)
                out_rows = out[e, 1:257, :].rearrange("(n p) d -> p n d", n=2)
                nc.sync.dma_start(out_rows, o_big[:])

                # ---- row 0 ----
                ui = pool.tile([1, D], F32, tag="ui")
                nc.vector.tensor_mul(ui[:], elnU.bitcast(F32)[0:1, :], iid_row[:])
                uo = pool.tile([1, D], F32, tag="uo")
                _lrelu(nc, uo[:], ui[:])
                nc.sync.dma_start(out[e, 0:1, :], uo[:])

    nc.compile()
    return nc


def _host_consts(Wa, ba, a_w, a_b):
    aq, ak, ai = a_w[:D], a_w[D:2 * D], a_w[2 * D:]
    vq = aq @ Wa
    vk = ak @ Wa
    vi = ai @ Wa
    c0 = float(ba @ aq + ba @ ak + ba @ ai + a_b[0])
    cstT = np.stack([vq, vk], axis=1).astype(np.float32)
    cstR = np.zeros((4, D), np.float32)
    cstR[0] = vi
    cstR[3, 0] = c0
    return cstT, cstR


_NC_CACHE = {}


def _get_nc():
    if "nc" not in _NC_CACHE:
        _NC_CACHE["nc"] = build()
    return _NC_CACHE["nc"]


def run(embeddings, Wa, ba, a_w, a_b, ln_g, ln_b, **spmd_kwargs):
    embeddings = np.ascontiguousarray(embeddings, dtype=np.float32)
    cstT, cstR = _host_consts(np.asarray(Wa, np.float32), np.asarray(ba, np.float32),
                              np.asarray(a_w, np.float32), np.asarray(a_b, np.float32))
    cstR[1] = np.asarray(ln_g, np.float32)
    cstR[2] = np.asarray(ln_b, np.float32)

    nc = _get_nc()
    in_maps = [
        {"emb": embeddings[c * B_LOC:(c + 1) * B_LOC], "cstT": cstT, "cstR": cstR}
        for c in range(N_CORES)
    ]
    res = run_bass_kernel_spmd(nc, in_maps, core_ids=list(range(N_CORES)), **spmd_kwargs)
    outp = np.concatenate([res.results[c]["out"] for c in range(N_CORES)], axis=0)
    return outp, res


def kernel(embeddings, Wa, ba, a_w, a_b, ln_g, ln_b):
    outp, _ = run(embeddings, Wa, ba, a_w, a_b, ln_g, ln_b)
    return outp
